# revision 8
# baseline (speedup 1.0000x reference)
"""KeyedLSTM Trainium2 kernel.

Strategy: tensor-parallel split of the 4H gate dimension across 8 cores.
Each core holds W_ih/W_hh column slices for its 256 h-rows (of each of the
g/i/f/o gate blocks) resident in SBUF, computes its slice of the gates /
c / h each step, and an AllGather of h (2048x32 fp16) runs every step so
every core has the full h for the next step's recurrent matmul.

v2 changes over the first working version:
- The x @ W_ih.T + bias precompute is folded INTO the recurrence loop: at
  step t the core computes the xw chunk for step t+L directly into the
  PSUM accumulator tile that step t+L's recurrent matmuls then accumulate
  into (bias rides a padded ones-row k-tile of x / bias-row of W_ih).
  This deletes the bulk phase-1 pass, the xw DRAM round trip, and the
  per-step gpre add, and gives the PE real work during the collective.
- Single merged activation chain on [128, 8, 32] views (one PSUM tile per
  step) instead of two per-sub-block chains: ~7 serialized ops.
- The kernel output tensor is fp16 (host upcasts); out DMA reads the same
  tile that feeds the AllGather.
- The ag_out -> h_sb gather DMA is split across two engine queues.

Precision: recurrent matmul in fp16 (weights + h) with fp32 PSUM
accumulation; x-part matmul also fp16. c and the gates stay fp32.
"""

import os
import sys

import numpy as np

for _p in (
    "/root/.axon_site",
    "/root/.axon_site/_ro/trn_rl_repo",
    "/root/.axon_site/_ro/pypackages",
    "/opt/trn_rl_repo",
):
    if os.path.isdir(_p) and _p not in sys.path:
        sys.path.append(_p)

import concourse.bacc as bacc
import concourse.bass_utils as bass_utils
import concourse.mybir as mybir
import concourse.tile as tile

AF = mybir.ActivationFunctionType
ALU = mybir.AluOpType
DT = mybir.dt

B, S, I, H = 32, 256, 1024, 2048
KB, KL = 4, 16
NCORES = 8
HLOC = H // NCORES  # 256 h rows per core
MT = 8  # m-tiles of 128 gate rows per core
KT_I = I // 128  # 8
KT_IP = KT_I + 1  # +1 padded k-tile carrying the ones-row (bias)
KT_H = H // 128  # 16
LOOKAHEAD = 2  # xw chunks computed this many steps ahead

_GOFF = {"i": 0, "f": H, "g": 2 * H, "o": 3 * H}
# m-tile order within each 128-row sub-block: tanh gate first (index 0) so
# sigmoid gates are a contiguous [1:4] slice.
_ORDER = ("g", "i", "f", "o")


def _rows_for_core(j):
    rows = []
    for p in range(2):
        base = j * HLOC + p * 128
        for g in _ORDER:
            o = _GOFF[g] + base
            rows.extend(range(o, o + 128))
    return np.asarray(rows, dtype=np.int64)


def _build_program(s_steps):
    """One SPMD program, identical on all cores; weights differ per-core."""
    ttok = s_steps * B

    nc = bacc.Bacc(
        "TRN2",
        target_bir_lowering=False,
        debug=False,
        enable_asserts=False,
        num_devices=NCORES,
    )

    # x (+ones row) as fp16, token-major columns [(t, b)]
    xt = nc.dram_tensor("xt", [KT_IP * 128, ttok], DT.float16, kind="ExternalInput").ap()
    kt = nc.dram_tensor("kt", [KT_IP * 128, KL * KB], DT.float16, kind="ExternalInput").ap()
    wih = nc.dram_tensor("wih", [KT_IP * 128, MT * 128], DT.float16, kind="ExternalInput").ap()
    whh = nc.dram_tensor("whh", [H, MT * 128], DT.float16, kind="ExternalInput").ap()
    out = nc.dram_tensor(
        "out", [s_steps, 2 * 128, B], DT.float16, kind="ExternalOutput"
    ).ap()

    rg = [list(range(NCORES))]

    with tile.TileContext(nc) as tc:
        with (
            tc.tile_pool(name="const", bufs=1) as const_pool,
            tc.tile_pool(name="xring", bufs=LOOKAHEAD + 3) as x_pool,
            tc.tile_pool(name="xwps", bufs=LOOKAHEAD + 1, space="PSUM") as xw_pool,
            tc.tile_pool(name="mtmp", bufs=2) as mtmp_pool,
            tc.tile_pool(name="mdram", bufs=2, space="DRAM") as mdram_pool,
        ):
            # ---- resident tensors ----
            whh_sb = const_pool.tile([128, KT_H, MT, 128], DT.float16)
            whh_re = whh.rearrange("(k p) m -> p k m", p=128)
            for g in range(4):
                nc.sync.dma_start(
                    whh_sb[:, 4 * g : 4 * (g + 1), :, :],
                    whh_re[:, 4 * g : 4 * (g + 1), :],
                )
            wih_sb = const_pool.tile([128, KT_IP, MT, 128], DT.float16)
            wih_re = wih.rearrange("(k p) m -> p k m", p=128)
            for g in range(3):
                nc.sync.dma_start(
                    wih_sb[:, 3 * g : 3 * (g + 1), :, :],
                    wih_re[:, 3 * g : 3 * (g + 1), :],
                )

            h_sb = const_pool.tile([128, KT_H, B], DT.float16)
            c_sb = const_pool.tile([128, 2, B], DT.float32)
            hk_sb = const_pool.tile([128, KT_H, KB], DT.float16)
            ck_sb = const_pool.tile([128, 2, KB], DT.float32)
            mult_sb = const_pool.tile([128, KL, 2], DT.float32)
            xk_sb = const_pool.tile([128, MT, KL * KB], DT.float32)
            nc.vector.memset(h_sb[:], 0.0)
            nc.vector.memset(c_sb[:], 0.0)
            nc.vector.memset(hk_sb[:], 0.0)
            nc.vector.memset(ck_sb[:], 0.0)

            # ---- key-seq x-part (tiny, kept in SBUF; includes bias) ----
            with (
                tc.tile_pool(name="kxin", bufs=1) as kxin_pool,
                tc.tile_pool(name="kxps", bufs=1, space="PSUM") as kxps_pool,
            ):
                k_sb = kxin_pool.tile([128, KT_IP, KL * KB], DT.float16, tag="kin")
                nc.sync.dma_start(k_sb[:], kt.rearrange("(k p) t -> p k t", p=128))
                kps = kxps_pool.tile([128, KL * KB], DT.float32, tag="kps")
                for m in range(MT):
                    for k in range(KT_IP):
                        nc.tensor.matmul(
                            kps[:],
                            wih_sb[:, k, m, :],
                            k_sb[:, k, :],
                            start=(k == 0),
                            stop=(k == KT_IP - 1),
                            skip_group_check=True,
                        )
                    nc.scalar.activation(xk_sb[:, m, :], kps[:], AF.Identity)

            # ---- xw lookahead priming for main steps 0..L-1 ----
            xw_tiles = {}
            xt_re = xt.rearrange("(k p) t -> p k t", p=128)

            def xw_chunk(t):
                x_t = x_pool.tile([128, KT_IP, B], DT.float16, tag="xin", name=f"x_{t}")
                nc.scalar.dma_start(x_t[:], xt_re[:, :, t * B : (t + 1) * B])
                ps = xw_pool.tile(
                    [128, MT, B],
                    DT.float32,
                    tag="xw",
                    name=f"xw_{t}",
                    padded_shape=[128, 2 * MT, B],  # own a full 2KB zero region
                )
                xw_tiles[t] = ps
                for m in range(MT):
                    for k in range(KT_IP):
                        # start=True only on the very first matmul: it marks the
                        # whole zero region pending-zero; each m's first write
                        # then overwrites its own bytes and later writes (incl.
                        # the recurrent matmuls at step t) accumulate.
                        nc.tensor.matmul(
                            ps[:, m, :],
                            wih_sb[:, k, m, :],
                            x_t[:, k, :],
                            start=(m == 0 and k == 0),
                            stop=False,
                            skip_group_check=True,
                        )

            for t in range(min(LOOKAHEAD, s_steps)):
                xw_chunk(t)

            # ---- key recurrence (collect forget-gate means) ----
            with (
                tc.tile_pool(name="kps", bufs=2, space="PSUM") as kps_pool,
                tc.tile_pool(name="ktmp", bufs=2) as ktmp_pool,
                tc.tile_pool(name="kdram", bufs=2, space="DRAM") as kdram_pool,
            ):
                for t in range(KL):
                    ps = kps_pool.tile([128, MT, KB], DT.float32, tag="kps")
                    for m in range(MT):
                        for k in range(KT_H):
                            nc.tensor.matmul(
                                ps[:, m, :],
                                whh_sb[:, k, m, :],
                                hk_sb[:, k, :],
                                start=(k == 0),
                                stop=(k == KT_H - 1),
                                skip_group_check=True,
                            )
                    gpre = ktmp_pool.tile([128, MT, KB], DT.float32, tag="gpre")
                    nc.vector.tensor_add(
                        gpre[:], ps[:], xk_sb[:, :, t * KB : (t + 1) * KB]
                    )
                    gact = ktmp_pool.tile([128, MT, KB], DT.float32, tag="gact")
                    gpre_v = gpre.rearrange("p (s g) b -> p s g b", s=2)
                    gact_v = gact.rearrange("p (s g) b -> p s g b", s=2)
                    nc.scalar.activation(
                        gact_v[:, :, 1:4, :], gpre_v[:, :, 1:4, :], AF.Sigmoid
                    )
                    nc.scalar.activation(
                        gact_v[:, :, 0, :], gpre_v[:, :, 0, :], AF.Tanh
                    )
                    g_v = gact_v[:, :, 0, :]
                    i_v = gact_v[:, :, 1, :]
                    f_v = gact_v[:, :, 2, :]
                    o_v = gact_v[:, :, 3, :]
                    t1 = ktmp_pool.tile([128, 2, KB], DT.float32, tag="t1")
                    t2 = ktmp_pool.tile([128, 2, KB], DT.float32, tag="t2")
                    nc.vector.tensor_mul(t1[:], i_v, g_v)
                    nc.vector.tensor_mul(t2[:], f_v, ck_sb[:])
                    nc.vector.tensor_add(ck_sb[:], t1[:], t2[:])
                    fs = ktmp_pool.tile([128, 2], DT.float32, tag="fs")
                    nc.vector.tensor_reduce(fs[:], f_v, mybir.AxisListType.X, ALU.add)
                    nc.vector.tensor_scalar_mul(mult_sb[:, t, :], fs[:], 1.0 / KB)
                    th = ktmp_pool.tile([128, 2, KB], DT.float32, tag="th")
                    nc.scalar.activation(th[:], ck_sb[:], AF.Tanh)
                    hloc = ktmp_pool.tile([128, 2, KB], DT.float16, tag="hloc")
                    nc.vector.tensor_mul(hloc[:], o_v, th[:])
                    ag_in = kdram_pool.tile([2 * 128, KB], DT.float16, tag="agin")
                    nc.sync.dma_start(
                        ag_in.rearrange("(s p) b -> p s b", p=128), hloc[:]
                    )
                    ag_out = kdram_pool.tile(
                        [H, KB], DT.float16, tag="agout", addr_space="Shared"
                    )
                    nc.gpsimd.collective_compute(
                        "AllGather",
                        ALU.bypass,
                        replica_groups=rg,
                        ins=[ag_in.opt()],
                        outs=[ag_out.opt()],
                    )
                    nc.sync.dma_start(
                        hk_sb[:], ag_out.rearrange("(k p) b -> p k b", p=128)
                    )

            # ---- main recurrence ----
            for t in range(s_steps):
                ps = xw_tiles.pop(t)
                for k in range(KT_H):
                    for m in range(MT):
                        nc.tensor.matmul(
                            ps[:, m, :],
                            whh_sb[:, k, m, :],
                            h_sb[:, k, :],
                            start=False,
                            stop=(k == KT_H - 1 and m == MT - 1),
                            skip_group_check=True,
                        )


                # merged activation chain on [128, (s g), B] views
                ps_v = ps.rearrange("p (s g) b -> p s g b", s=2)
                gact = mtmp_pool.tile([128, 2, 4, B], DT.float32, tag="gact")
                nc.scalar.activation(gact[:, :, 0, :], ps_v[:, :, 0, :], AF.Tanh)
                nc.scalar.activation(gact[:, :, 1:4, :], ps_v[:, :, 1:4, :], AF.Sigmoid)
                g_v = gact[:, :, 0, :]
                i_v = gact[:, :, 1, :]
                f_v = gact[:, :, 2, :]
                o_v = gact[:, :, 3, :]
                t1 = mtmp_pool.tile([128, 2, B], DT.float32, tag="t1")
                t2 = mtmp_pool.tile([128, 2, B], DT.float32, tag="t2")
                nc.vector.tensor_mul(t1[:], i_v, g_v)
                nc.vector.tensor_mul(t2[:], f_v, c_sb[:])
                nc.vector.tensor_add(c_sb[:], t1[:], t2[:])
                th = mtmp_pool.tile([128, 2, B], DT.float32, tag="th")
                nc.scalar.activation(th[:], c_sb[:], AF.Tanh)
                hsend = mtmp_pool.tile([128, 2, B], DT.float16, tag="hsend")
                nc.vector.tensor_mul(hsend[:], o_v, th[:])
                if t + LOOKAHEAD < s_steps:
                    xw_chunk(t + LOOKAHEAD)
                # output = h BEFORE key gating
                nc.scalar.dma_start(
                    out[t].rearrange("(s p) b -> p s b", p=128), hsend[:]
                )
                if t < KL:
                    for s_ in range(2):
                        nc.vector.tensor_scalar_mul(
                            hsend[:, s_, :], hsend[:, s_, :], mult_sb[:, t, s_ : s_ + 1]
                        )
                        nc.vector.tensor_scalar_mul(
                            c_sb[:, s_, :], c_sb[:, s_, :], mult_sb[:, t, s_ : s_ + 1]
                        )

                if t == s_steps - 1:
                    break  # last h never consumed
                ag_in = mdram_pool.tile([2 * 128, B], DT.float16, tag="agin")
                ag_in_re = ag_in.rearrange("(s p) b -> p s b", p=128)
                nc.sync.dma_start(ag_in_re[:, 0, :], hsend[:, 0, :])
                nc.gpsimd.dma_start(ag_in_re[:, 1, :], hsend[:, 1, :])
                ag_out = mdram_pool.tile(
                    [H, B], DT.float16, tag="agout", addr_space="Shared"
                )
                nc.gpsimd.collective_compute(
                    "AllGather",
                    ALU.bypass,
                    replica_groups=rg,
                    ins=[ag_in.opt()],
                    outs=[ag_out.opt()],
                )
                ag_re = ag_out.rearrange("(k p) b -> p k b", p=128)
                nc.sync.dma_start(h_sb[:, 0:4, :], ag_re[:, 0:4, :])
                nc.scalar.dma_start(h_sb[:, 4:8, :], ag_re[:, 4:8, :])
                nc.gpsimd.dma_start(h_sb[:, 8:12, :], ag_re[:, 8:12, :])
                nc.sync.dma_start(h_sb[:, 12:16, :], ag_re[:, 12:16, :])

    nc.compile()
    return nc


def _prepare_inputs(x, key_seq, weight_ih, weight_hh, bias_ih, bias_hh, s_steps):
    x = np.ascontiguousarray(np.asarray(x, dtype=np.float32)[:, :s_steps, :])
    key_seq = np.asarray(key_seq, dtype=np.float32)
    weight_ih = np.asarray(weight_ih, dtype=np.float32)
    weight_hh = np.asarray(weight_hh, dtype=np.float32)
    b = (np.asarray(bias_ih, dtype=np.float32) + np.asarray(bias_hh, dtype=np.float32))
    ttok = s_steps * B

    # tokens ordered (s, b): column s*B + b; pad with a ones-row k-tile
    xt = np.zeros((KT_IP * 128, ttok), np.float16)
    xt[:I] = x.transpose(2, 1, 0).reshape(I, ttok).astype(np.float16)
    xt[I] = 1.0
    kt = np.zeros((KT_IP * 128, KL * KB), np.float16)
    kt[:I] = key_seq.transpose(2, 1, 0).reshape(I, KL * KB).astype(np.float16)
    kt[I] = 1.0

    in_maps = []
    for j in range(NCORES):
        rows = _rows_for_core(j)
        wih_j = np.zeros((KT_IP * 128, MT * 128), np.float16)
        wih_j[:I] = weight_ih[rows].T.astype(np.float16)
        wih_j[I] = b[rows].astype(np.float16)
        in_maps.append(
            {
                "xt": xt,
                "kt": kt,
                "wih": wih_j,
                "whh": np.ascontiguousarray(weight_hh[rows].T.astype(np.float16)),
            }
        )
    return in_maps


_NC_CACHE = {}


def _run(x, key_seq, weight_ih, weight_hh, bias_ih, bias_hh, s_steps, trace=False):
    if s_steps not in _NC_CACHE:
        _NC_CACHE[s_steps] = _build_program(s_steps)
    nc = _NC_CACHE[s_steps]
    in_maps = _prepare_inputs(
        x, key_seq, weight_ih, weight_hh, bias_ih, bias_hh, s_steps
    )
    res = bass_utils.run_bass_kernel_spmd(
        nc, in_maps, core_ids=list(range(NCORES)), trace=trace
    )
    # out_j: [s, r, b] with global h row = j*HLOC + r
    pieces = [
        res.results[j]["out"].astype(np.float32).transpose(0, 2, 1)
        for j in range(NCORES)
    ]
    full = np.concatenate(pieces, axis=2)  # (s_steps, B, H)
    return full, res


def kernel(x, key_seq, weight_ih, weight_hh, bias_ih, bias_hh):
    s_steps = int(os.environ.get("KEYED_LSTM_STEPS", S))
    trace = os.environ.get("KEYED_LSTM_TRACE", "0") == "1"
    full, _res = _run(
        x, key_seq, weight_ih, weight_hh, bias_ih, bias_hh, s_steps, trace=trace
    )
    return full


# revision 9
# speedup vs baseline: 1.1508x; 1.1508x over previous
"""KeyedLSTM Trainium2 kernel.

Strategy: tensor-parallel split of the 4H gate dimension across 8 cores.
Each core holds W_ih/W_hh column slices for its 256 h-rows (of each of the
g/i/f/o gate blocks) resident in SBUF, computes its slice of the gates /
c / h each step, and an AllGather of h (2048x32 fp16) runs every step so
every core has the full h for the next step's recurrent matmul.

v2 changes over the first working version:
- The x @ W_ih.T + bias precompute is folded INTO the recurrence loop: at
  step t the core computes the xw chunk for step t+L directly into the
  PSUM accumulator tile that step t+L's recurrent matmuls then accumulate
  into (bias rides a padded ones-row k-tile of x / bias-row of W_ih).
  This deletes the bulk phase-1 pass, the xw DRAM round trip, and the
  per-step gpre add, and gives the PE real work during the collective.
- Single merged activation chain on [128, 8, 32] views (one PSUM tile per
  step) instead of two per-sub-block chains: ~7 serialized ops.
- The kernel output tensor is fp16 (host upcasts); out DMA reads the same
  tile that feeds the AllGather.
- The ag_out -> h_sb gather DMA is split across two engine queues.

Precision: recurrent matmul in fp16 (weights + h) with fp32 PSUM
accumulation; x-part matmul also fp16. c and the gates stay fp32.
"""

import os
import sys

import numpy as np

for _p in (
    "/root/.axon_site",
    "/root/.axon_site/_ro/trn_rl_repo",
    "/root/.axon_site/_ro/pypackages",
    "/opt/trn_rl_repo",
):
    if os.path.isdir(_p) and _p not in sys.path:
        sys.path.append(_p)

import concourse.bacc as bacc
import concourse.bass_utils as bass_utils
import concourse.mybir as mybir
import concourse.tile as tile

AF = mybir.ActivationFunctionType
ALU = mybir.AluOpType
DT = mybir.dt

B, S, I, H = 32, 256, 1024, 2048
KB, KL = 4, 16
NCORES = 8
HLOC = H // NCORES  # 256 h rows per core
MT = 8  # m-tiles of 128 gate rows per core
KT_I = I // 128  # 8
KT_IP = KT_I + 1  # +1 padded k-tile carrying the ones-row (bias)
KT_H = H // 128  # 16
LOOKAHEAD = 2  # xw chunks computed this many steps ahead

_GOFF = {"i": 0, "f": H, "g": 2 * H, "o": 3 * H}
# m-tile order within each 128-row sub-block: tanh gate first (index 0) so
# sigmoid gates are a contiguous [1:4] slice.
_ORDER = ("g", "i", "f", "o")


def _rows_for_core(j):
    rows = []
    for p in range(2):
        base = j * HLOC + p * 128
        for g in _ORDER:
            o = _GOFF[g] + base
            rows.extend(range(o, o + 128))
    return np.asarray(rows, dtype=np.int64)


def _build_program(s_steps):
    """One SPMD program, identical on all cores; weights differ per-core."""
    ttok = s_steps * B

    nc = bacc.Bacc(
        "TRN2",
        target_bir_lowering=False,
        debug=False,
        enable_asserts=False,
        num_devices=NCORES,
    )

    # x (+ones row) as fp16, token-major columns [(t, b)]
    xt = nc.dram_tensor("xt", [KT_IP * 128, ttok], DT.float16, kind="ExternalInput").ap()
    kt = nc.dram_tensor("kt", [KT_IP * 128, KL * KB], DT.float16, kind="ExternalInput").ap()
    wih = nc.dram_tensor("wih", [KT_IP * 128, MT * 128], DT.float16, kind="ExternalInput").ap()
    whh = nc.dram_tensor("whh", [H, MT * 128], DT.float16, kind="ExternalInput").ap()
    out = nc.dram_tensor(
        "out", [s_steps, 2 * 128, B], DT.float16, kind="ExternalOutput"
    ).ap()

    rg = [list(range(NCORES))]

    with tile.TileContext(nc) as tc:
        with (
            tc.tile_pool(name="const", bufs=1) as const_pool,
            tc.tile_pool(name="xring", bufs=LOOKAHEAD + 3) as x_pool,
            tc.tile_pool(name="xwps", bufs=LOOKAHEAD + 1, space="PSUM") as xw_pool,
            tc.tile_pool(name="mtmp", bufs=2) as mtmp_pool,
            tc.tile_pool(name="mdram", bufs=2, space="DRAM") as mdram_pool,
        ):
            # ---- resident tensors ----
            whh_sb = const_pool.tile([128, KT_H, MT, 128], DT.float16)
            whh_re = whh.rearrange("(k p) m -> p k m", p=128)
            for g in range(4):
                nc.sync.dma_start(
                    whh_sb[:, 4 * g : 4 * (g + 1), :, :],
                    whh_re[:, 4 * g : 4 * (g + 1), :],
                )
            wih_sb = const_pool.tile([128, KT_IP, MT, 128], DT.float16)
            wih_re = wih.rearrange("(k p) m -> p k m", p=128)
            for g in range(3):
                nc.sync.dma_start(
                    wih_sb[:, 3 * g : 3 * (g + 1), :, :],
                    wih_re[:, 3 * g : 3 * (g + 1), :],
                )

            h_sb = const_pool.tile([128, KT_H, B], DT.float16)
            c_sb = const_pool.tile([128, 2, B], DT.float32)
            hk_sb = const_pool.tile([128, KT_H, KB], DT.float16)
            ck_sb = const_pool.tile([128, 2, KB], DT.float32)
            mult_sb = const_pool.tile([128, KL, 2], DT.float32)
            xk_sb = const_pool.tile([128, MT, KL * KB], DT.float32)
            nc.vector.memset(h_sb[:], 0.0)
            nc.vector.memset(c_sb[:], 0.0)
            nc.vector.memset(hk_sb[:], 0.0)
            nc.vector.memset(ck_sb[:], 0.0)

            # ---- key-seq x-part (tiny, kept in SBUF; includes bias) ----
            with (
                tc.tile_pool(name="kxin", bufs=1) as kxin_pool,
                tc.tile_pool(name="kxps", bufs=1, space="PSUM") as kxps_pool,
            ):
                k_sb = kxin_pool.tile([128, KT_IP, KL * KB], DT.float16, tag="kin")
                nc.sync.dma_start(k_sb[:], kt.rearrange("(k p) t -> p k t", p=128))
                kps = kxps_pool.tile([128, KL * KB], DT.float32, tag="kps")
                for m in range(MT):
                    for k in range(KT_IP):
                        nc.tensor.matmul(
                            kps[:],
                            wih_sb[:, k, m, :],
                            k_sb[:, k, :],
                            start=(k == 0),
                            stop=(k == KT_IP - 1),
                            skip_group_check=True,
                        )
                    nc.scalar.activation(xk_sb[:, m, :], kps[:], AF.Identity)

            # ---- xw lookahead priming for main steps 0..L-1 ----
            xw_tiles = {}
            xt_re = xt.rearrange("(k p) t -> p k t", p=128)

            def xw_chunk(t):
                x_t = x_pool.tile([128, KT_IP, B], DT.float16, tag="xin", name=f"x_{t}")
                nc.scalar.dma_start(x_t[:], xt_re[:, :, t * B : (t + 1) * B])
                ps = xw_pool.tile(
                    [128, MT, B],
                    DT.float32,
                    tag="xw",
                    name=f"xw_{t}",
                    padded_shape=[128, 2 * MT, B],  # own a full 2KB zero region
                )
                xw_tiles[t] = ps
                for m in range(MT):
                    for k in range(KT_IP):
                        # start=True only on the very first matmul: it marks the
                        # whole zero region pending-zero; each m's first write
                        # then overwrites its own bytes and later writes (incl.
                        # the recurrent matmuls at step t) accumulate.
                        nc.tensor.matmul(
                            ps[:, m, :],
                            wih_sb[:, k, m, :],
                            x_t[:, k, :],
                            start=(m == 0 and k == 0),
                            stop=False,
                            skip_group_check=True,
                        )

            for t in range(min(LOOKAHEAD, s_steps)):
                xw_chunk(t)

            # ---- key recurrence (collect forget-gate means) ----
            with (
                tc.tile_pool(name="kps", bufs=2, space="PSUM") as kps_pool,
                tc.tile_pool(name="ktmp", bufs=2) as ktmp_pool,
                tc.tile_pool(name="kdram", bufs=2, space="DRAM") as kdram_pool,
            ):
                for t in range(KL):
                    ps = kps_pool.tile([128, MT, KB], DT.float32, tag="kps")
                    for m in range(MT):
                        for k in range(KT_H):
                            nc.tensor.matmul(
                                ps[:, m, :],
                                whh_sb[:, k, m, :],
                                hk_sb[:, k, :],
                                start=(k == 0),
                                stop=(k == KT_H - 1),
                                skip_group_check=True,
                            )
                    gpre = ktmp_pool.tile([128, MT, KB], DT.float32, tag="gpre")
                    nc.vector.tensor_add(
                        gpre[:], ps[:], xk_sb[:, :, t * KB : (t + 1) * KB]
                    )
                    gact = ktmp_pool.tile([128, MT, KB], DT.float32, tag="gact")
                    gpre_v = gpre.rearrange("p (s g) b -> p s g b", s=2)
                    gact_v = gact.rearrange("p (s g) b -> p s g b", s=2)
                    nc.scalar.activation(
                        gact_v[:, :, 1:4, :], gpre_v[:, :, 1:4, :], AF.Sigmoid
                    )
                    nc.scalar.activation(
                        gact_v[:, :, 0, :], gpre_v[:, :, 0, :], AF.Tanh
                    )
                    g_v = gact_v[:, :, 0, :]
                    i_v = gact_v[:, :, 1, :]
                    f_v = gact_v[:, :, 2, :]
                    o_v = gact_v[:, :, 3, :]
                    t1 = ktmp_pool.tile([128, 2, KB], DT.float32, tag="t1")
                    t2 = ktmp_pool.tile([128, 2, KB], DT.float32, tag="t2")
                    nc.vector.tensor_mul(t1[:], i_v, g_v)
                    nc.vector.tensor_mul(t2[:], f_v, ck_sb[:])
                    nc.vector.tensor_add(ck_sb[:], t1[:], t2[:])
                    fs = ktmp_pool.tile([128, 2], DT.float32, tag="fs")
                    nc.vector.tensor_reduce(fs[:], f_v, mybir.AxisListType.X, ALU.add)
                    nc.vector.tensor_scalar_mul(mult_sb[:, t, :], fs[:], 1.0 / KB)
                    th = ktmp_pool.tile([128, 2, KB], DT.float32, tag="th")
                    nc.scalar.activation(th[:], ck_sb[:], AF.Tanh)
                    hloc = ktmp_pool.tile([128, 2, KB], DT.float16, tag="hloc")
                    nc.vector.tensor_mul(hloc[:], o_v, th[:])
                    ag_in = kdram_pool.tile([2 * 128, KB], DT.float16, tag="agin")
                    nc.sync.dma_start(
                        ag_in.rearrange("(p s) b -> p s b", p=128), hloc[:]
                    )
                    ag_out = kdram_pool.tile(
                        [H, KB], DT.float16, tag="agout", addr_space="Shared"
                    )
                    nc.gpsimd.collective_compute(
                        "AllGather",
                        ALU.bypass,
                        replica_groups=rg,
                        ins=[ag_in.opt()],
                        outs=[ag_out.opt()],
                    )
                    nc.sync.dma_start(
                        hk_sb.rearrange("p (j s) b -> p j s b", s=2),
                        ag_out.rearrange("(j p s) b -> p j s b", p=128, s=2),
                    )

            # ---- main recurrence ----
            for t in range(s_steps):
                ps = xw_tiles.pop(t)
                for k in range(KT_H):
                    for m in range(MT):
                        nc.tensor.matmul(
                            ps[:, m, :],
                            whh_sb[:, k, m, :],
                            h_sb[:, k, :],
                            start=False,
                            stop=(k == KT_H - 1 and m == MT - 1),
                            skip_group_check=True,
                        )


                # merged activation chain on [128, (s g), B] views
                ps_v = ps.rearrange("p (s g) b -> p s g b", s=2)
                gact = mtmp_pool.tile([128, 2, 4, B], DT.float32, tag="gact")
                nc.scalar.activation(gact[:, :, 0, :], ps_v[:, :, 0, :], AF.Tanh)
                nc.scalar.activation(gact[:, :, 1:4, :], ps_v[:, :, 1:4, :], AF.Sigmoid)
                g_v = gact[:, :, 0, :]
                i_v = gact[:, :, 1, :]
                f_v = gact[:, :, 2, :]
                o_v = gact[:, :, 3, :]
                t1 = mtmp_pool.tile([128, 2, B], DT.float32, tag="t1")
                t2 = mtmp_pool.tile([128, 2, B], DT.float32, tag="t2")
                nc.vector.tensor_mul(t1[:], i_v, g_v)
                nc.vector.tensor_mul(t2[:], f_v, c_sb[:])
                nc.vector.tensor_add(c_sb[:], t1[:], t2[:])
                th = mtmp_pool.tile([128, 2, B], DT.float32, tag="th")
                nc.scalar.activation(th[:], c_sb[:], AF.Tanh)
                hsend = mtmp_pool.tile([128, 2, B], DT.float16, tag="hsend")
                nc.vector.tensor_mul(hsend[:], o_v, th[:])
                if t + LOOKAHEAD < s_steps:
                    xw_chunk(t + LOOKAHEAD)
                # output = h BEFORE key gating
                nc.scalar.dma_start(
                    out[t].rearrange("(s p) b -> p s b", p=128), hsend[:]
                )
                if t < KL:
                    for s_ in range(2):
                        nc.vector.tensor_scalar_mul(
                            hsend[:, s_, :], hsend[:, s_, :], mult_sb[:, t, s_ : s_ + 1]
                        )
                        nc.vector.tensor_scalar_mul(
                            c_sb[:, s_, :], c_sb[:, s_, :], mult_sb[:, t, s_ : s_ + 1]
                        )

                if t == s_steps - 1:
                    break  # last h never consumed
                ag_in = mdram_pool.tile([2 * 128, B], DT.float16, tag="agin")
                nc.sync.dma_start(
                    ag_in.rearrange("(p s) b -> p s b", p=128), hsend[:]
                )
                ag_out = mdram_pool.tile(
                    [H, B], DT.float16, tag="agout", addr_space="Shared"
                )
                nc.gpsimd.collective_compute(
                    "AllGather",
                    ALU.bypass,
                    replica_groups=rg,
                    ins=[ag_in.opt()],
                    outs=[ag_out.opt()],
                )
                ag_re = ag_out.rearrange("(j p s) b -> p j s b", p=128, s=2)
                h_v = h_sb.rearrange("p (j s) b -> p j s b", s=2)
                nc.sync.dma_start(h_v[:, 0:4, :, :], ag_re[:, 0:4, :, :])
                nc.gpsimd.dma_start(h_v[:, 4:8, :, :], ag_re[:, 4:8, :, :])

    nc.compile()
    return nc


def _prepare_inputs(x, key_seq, weight_ih, weight_hh, bias_ih, bias_hh, s_steps):
    x = np.ascontiguousarray(np.asarray(x, dtype=np.float32)[:, :s_steps, :])
    key_seq = np.asarray(key_seq, dtype=np.float32)
    weight_ih = np.asarray(weight_ih, dtype=np.float32)
    weight_hh = np.asarray(weight_hh, dtype=np.float32)
    b = (np.asarray(bias_ih, dtype=np.float32) + np.asarray(bias_hh, dtype=np.float32))
    ttok = s_steps * B

    # tokens ordered (s, b): column s*B + b; pad with a ones-row k-tile
    xt = np.zeros((KT_IP * 128, ttok), np.float16)
    xt[:I] = x.transpose(2, 1, 0).reshape(I, ttok).astype(np.float16)
    xt[I] = 1.0
    kt = np.zeros((KT_IP * 128, KL * KB), np.float16)
    kt[:I] = key_seq.transpose(2, 1, 0).reshape(I, KL * KB).astype(np.float16)
    kt[I] = 1.0

    in_maps = []
    for j in range(NCORES):
        rows = _rows_for_core(j)
        wih_j = np.zeros((KT_IP * 128, MT * 128), np.float16)
        wih_j[:I] = weight_ih[rows].T.astype(np.float16)
        wih_j[I] = b[rows].astype(np.float16)
        in_maps.append(
            {
                "xt": xt,
                "kt": kt,
                "wih": wih_j,
                "whh": np.ascontiguousarray(weight_hh[rows].T.astype(np.float16)),
            }
        )
    return in_maps


_NC_CACHE = {}


def _run(x, key_seq, weight_ih, weight_hh, bias_ih, bias_hh, s_steps, trace=False):
    if s_steps not in _NC_CACHE:
        _NC_CACHE[s_steps] = _build_program(s_steps)
    nc = _NC_CACHE[s_steps]
    in_maps = _prepare_inputs(
        x, key_seq, weight_ih, weight_hh, bias_ih, bias_hh, s_steps
    )
    res = bass_utils.run_bass_kernel_spmd(
        nc, in_maps, core_ids=list(range(NCORES)), trace=trace
    )
    # out_j: [s, r, b] with global h row = j*HLOC + r
    pieces = [
        res.results[j]["out"].astype(np.float32).transpose(0, 2, 1)
        for j in range(NCORES)
    ]
    full = np.concatenate(pieces, axis=2)  # (s_steps, B, H)
    return full, res


def kernel(x, key_seq, weight_ih, weight_hh, bias_ih, bias_hh):
    s_steps = int(os.environ.get("KEYED_LSTM_STEPS", S))
    trace = os.environ.get("KEYED_LSTM_TRACE", "0") == "1"
    full, _res = _run(
        x, key_seq, weight_ih, weight_hh, bias_ih, bias_hh, s_steps, trace=trace
    )
    return full
